# revision 12
# baseline (speedup 1.0000x reference)
"""Trainium2 Bass kernel for ChromophoreSolventGNN (2x GCNConv + BN + mean-pool + MLP head).

Strategy (8 NeuronCores, SPMD), v2:
  - Destination-shard nodes: core c owns contiguous node range [c*2560, (c+1)*2560)
    (N=20000 padded to 20480). Edges (incl. self-loops) routed to the owner of
    their destination (col), sorted by destination block (128 nodes).
  - Layer-1 edge source rows are PRE-GATHERED ON HOST (xg, fp16) - no device
    gather for layer 1. One-hot scatter matrices (norm folded in) and the
    mean-pool one-hot are also host-built and streamed from HBM, eliminating
    the DVE one-hot generation of v1.
  - Aggregation computes agg^T directly: matmul(lhsT=rows[128e,F], rhs=oh[128e,128d])
    -> psum [F, 128d]; feature-major feeds the projection without transposes.
  - Layer-2 gathers h1 rows from the AllGathered table via gpsimd dma_gather
    (small per-call descriptor prep, overlapped with compute via buffered pools).
  - BatchNorm is shift-invariant => conv biases drop out. Stats AllReduce'd.
  - Mean-pool via one-hot matmul; pooled sums AllReduce'd; small MLP head
    computed replicated on every core.
"""

import numpy as np

import concourse.bass as bass
import concourse.mybir as mybir
from concourse import bacc
from concourse.bass_utils import run_bass_kernel_spmd
from concourse.tile import TileContext

F32 = mybir.dt.float32
F16 = mybir.dt.float16
I16 = mybir.dt.int16
I32 = mybir.dt.int32

W = 8            # cores
N = 20000        # nodes
E = 320000       # edges
G = 512          # graphs
F_IN = 64
H1 = 128
H2 = 256
SOLV = 128
EPS = 1e-5

NB = 20                  # destination blocks of 128 nodes per core
PC = NB * 128            # nodes per core (2560)
NP = W * PC              # padded node count (20480)
CCH_MAX = 8              # chunks per gather call (<=1024 idxs, HW desc-ring limit)


def _bn_apply_params(nc, tl, st, colw, n_count, g_sb, be_sb, name):
    """From (sum, sumsq) slices compute per-partition scale/shift tiles."""
    mu = tl.tile([128, 1], F32, tag=f"mu{name}")
    nc.vector.tensor_scalar_mul(mu[:], st[:, colw : colw + 1], 1.0 / n_count)
    var = tl.tile([128, 1], F32, tag=f"var{name}")
    nc.vector.tensor_scalar_mul(var[:], st[:, colw + 1 : colw + 2], 1.0 / n_count)
    musq = tl.tile([128, 1], F32, tag=f"musq{name}")
    nc.vector.tensor_tensor(out=musq[:], in0=mu[:], in1=mu[:], op=mybir.AluOpType.mult)
    nc.vector.tensor_tensor(out=var[:], in0=var[:], in1=musq[:], op=mybir.AluOpType.subtract)
    nc.vector.tensor_scalar_add(var[:], var[:], EPS)
    rv = tl.tile([128, 1], F32, tag=f"rv{name}")
    nc.vector.reciprocal(out=rv[:], in_=var[:])
    rstd = tl.tile([128, 1], F32, tag=f"rstd{name}")
    nc.scalar.sqrt(out=rstd[:], in_=rv[:])
    sc = tl.tile([128, 1], F32, tag=f"sc{name}")
    nc.vector.tensor_tensor(out=sc[:], in0=g_sb[:], in1=rstd[:], op=mybir.AluOpType.mult)
    sh = tl.tile([128, 1], F32, tag=f"sh{name}")
    nc.vector.tensor_tensor(out=sh[:], in0=mu[:], in1=sc[:], op=mybir.AluOpType.mult)
    nc.vector.tensor_tensor(out=sh[:], in0=be_sb[:], in1=sh[:], op=mybir.AluOpType.subtract)
    return sc, sh


def _build_program(C):
    """Build the SPMD Bass program. C = chunks (of 128 edge slots) per dst block."""
    NC = NB * C           # chunks per core
    NE = NC * 128         # edge slots per core

    nc = bacc.Bacc("TRN2", target_bir_lowering=False, debug=False, num_devices=W)

    # ---- external inputs -------------------------------------------------
    xg_d = nc.dram_tensor("xg", [128, NC, F_IN], F16, kind="ExternalInput")
    oh_d = nc.dram_tensor("oh", [128, NC, 128], F16, kind="ExternalInput")
    gidx_d = nc.dram_tensor("gidx", [128, NE // 16], I16, kind="ExternalInput")
    idx2_d = nc.dram_tensor("idx2", [128, NC], I32, kind="ExternalInput")
    ohg_d = nc.dram_tensor("ohg", [128, NB, G], F16, kind="ExternalInput")
    id16_d = nc.dram_tensor("ident16", [128, 128], F16, kind="ExternalInput")
    sfT_d = nc.dram_tensor("sfT", [SOLV, G], F32, kind="ExternalInput")
    w1_d = nc.dram_tensor("w1", [F_IN, H1], F32, kind="ExternalInput")
    w2_d = nc.dram_tensor("w2", [H1, H2], F32, kind="ExternalInput")
    ws_d = nc.dram_tensor("ws", [SOLV, 128], F32, kind="ExternalInput")
    wf1_d = nc.dram_tensor("wf1", [128, 3, 128], F32, kind="ExternalInput")
    wf2_d = nc.dram_tensor("wf2", [128, 1], F32, kind="ExternalInput")
    g1_d = nc.dram_tensor("g1", [128, 1], F32, kind="ExternalInput")
    be1_d = nc.dram_tensor("be1", [128, 1], F32, kind="ExternalInput")
    g2_d = nc.dram_tensor("g2", [128, 2], F32, kind="ExternalInput")
    be2_d = nc.dram_tensor("be2", [128, 2], F32, kind="ExternalInput")
    gf1_d = nc.dram_tensor("gf1", [128, 1], F32, kind="ExternalInput")
    bef1_d = nc.dram_tensor("bef1", [128, 1], F32, kind="ExternalInput")
    bs_d = nc.dram_tensor("bs", [128, 1], F32, kind="ExternalInput")
    bf2_d = nc.dram_tensor("bf2", [1, 1], F32, kind="ExternalInput")

    out_d = nc.dram_tensor("out", [G, 1], F32, kind="ExternalOutput")

    # ---- internal DRAM ---------------------------------------------------
    h1loc_d = nc.dram_tensor("h1loc", [PC, H1], F16)
    h1full_d = nc.dram_tensor("h1full", [NP, H1], F16, addr_space="Shared")
    bn1i_d = nc.dram_tensor("bn1i", [128, 2], F32)
    bn1o_d = nc.dram_tensor("bn1o", [128, 2], F32, addr_space="Shared")
    bn2i_d = nc.dram_tensor("bn2i", [128, 4], F32)
    bn2o_d = nc.dram_tensor("bn2o", [128, 4], F32, addr_space="Shared")
    pli_d = nc.dram_tensor("pli", [2 * 128, G], F32)
    plo_d = nc.dram_tensor("plo", [2 * 128, G], F32, addr_space="Shared")

    RG = [list(range(W))]
    CCH = CCH_MAX
    while NC % CCH:
        CCH -= 1
    NGC = NC // CCH        # gather calls / stream groups per layer
    CPC = CCH * 128        # edge slots per gather call (<=1024)
    NKC = PC // 512        # 512-node column chunks

    with TileContext(nc) as tc:
        with tc.tile_pool(name="const", bufs=1) as cst, \
             tc.tile_pool(name="ohst", bufs=3) as ohp, \
             tc.tile_pool(name="ps", bufs=2, space="PSUM") as ps, \
             tc.tile_pool(name="psacc", bufs=2, space="PSUM") as psacc:

            # ---------- setup: constants ----------
            def load_const(name, dram, shape, dt):
                t = cst.tile(shape, dt, name=name)
                nc.sync.dma_start(out=t[:], in_=dram[:])
                return t

            gidx_sb = load_const("gidx_sb", gidx_d, [128, NE // 16], I16)
            idx2_sb = load_const("idx2_sb", idx2_d, [128, NC], I32)
            ohg_sb = load_const("ohg_sb", ohg_d, [128, NB, G], F16)
            id16_sb = load_const("id16_sb", id16_d, [128, 128], F16)
            sfT_sb = load_const("sfT_sb", sfT_d, [SOLV, G], F32)
            w1_sb = load_const("w1_sb", w1_d, [F_IN, H1], F32)
            w2_sb = load_const("w2_sb", w2_d, [H1, H2], F32)
            ws_sb = load_const("ws_sb", ws_d, [SOLV, 128], F32)
            wf1_sb = load_const("wf1_sb", wf1_d, [128, 3, 128], F32)
            wf2_sb = load_const("wf2_sb", wf2_d, [128, 1], F32)
            g1_sb = load_const("g1_sb", g1_d, [128, 1], F32)
            be1_sb = load_const("be1_sb", be1_d, [128, 1], F32)
            g2_sb = load_const("g2_sb", g2_d, [128, 2], F32)
            be2_sb = load_const("be2_sb", be2_d, [128, 2], F32)
            gf1_sb = load_const("gf1_sb", gf1_d, [128, 1], F32)
            bef1_sb = load_const("bef1_sb", bef1_d, [128, 1], F32)
            bs_sb = load_const("bs_sb", bs_d, [128, 1], F32)
            bf2_sb = load_const("bf2_sb", bf2_d, [1, 1], F32)

            # ---------- layer 1: agg^T = (Xg)^T-chunks @ oh-chunks ----------
            l1 = tc.alloc_tile_pool(name="l1", bufs=1)
            l1s = tc.alloc_tile_pool(name="l1s", bufs=3)
            aggT1 = l1.tile([F_IN, NB * 128], F32)
            acc = None
            for g in range(NGC):
                ohb = ohp.tile([128, CCH, 128], F16, tag="ohb")
                nc.sync.dma_start(out=ohb[:], in_=oh_d[:, g * CCH : (g + 1) * CCH, :])
                xgb = l1s.tile([128, CCH, F_IN], F16, tag="xgb")
                nc.sync.dma_start(out=xgb[:], in_=xg_d[:, g * CCH : (g + 1) * CCH, :])
                for jj in range(CCH):
                    j = g * CCH + jj
                    b, cidx = divmod(j, C)
                    if cidx == 0:
                        acc = psacc.tile([F_IN, 128], F32, tag="acc")
                    nc.tensor.matmul(
                        out=acc[:],
                        lhsT=xgb[:, jj, :],
                        rhs=ohb[:, jj, :],
                        start=(cidx == 0),
                        stop=(cidx == C - 1),
                    )
                    if cidx == C - 1:
                        nc.vector.tensor_copy(out=aggT1[:, b * 128 : (b + 1) * 128], in_=acc[:])

            # project: h1T [H1, PC] = W1.T @ aggT1 ; BN1 stats along nodes
            h1T_sb = l1.tile([H1, NB * 128], F32)
            s1p = l1.tile([128, 16], F32)
            for k in range(NKC):
                ph = ps.tile([H1, 512], F32, tag="ph")
                nc.tensor.matmul(
                    out=ph[:], lhsT=w1_sb[:],
                    rhs=aggT1[:, k * 512 : (k + 1) * 512],
                    start=True, stop=True,
                )
                nc.vector.tensor_copy(out=h1T_sb[:, k * 512 : (k + 1) * 512], in_=ph[:])
                nc.vector.tensor_reduce(
                    out=s1p[:, k : k + 1], in_=ph[:],
                    axis=mybir.AxisListType.X, op=mybir.AluOpType.add,
                )
                sq = l1s.tile([H1, 512], F32, tag="sq1")
                nc.scalar.square(out=sq[:], in_=ph[:])
                nc.vector.tensor_reduce(
                    out=s1p[:, NKC + k : NKC + k + 1], in_=sq[:],
                    axis=mybir.AxisListType.X, op=mybir.AluOpType.add,
                )
            st1 = l1.tile([128, 2], F32)
            nc.vector.tensor_reduce(out=st1[:, 0:1], in_=s1p[:, 0:NKC],
                                    axis=mybir.AxisListType.X, op=mybir.AluOpType.add)
            nc.vector.tensor_reduce(out=st1[:, 1:2], in_=s1p[:, NKC:2 * NKC],
                                    axis=mybir.AxisListType.X, op=mybir.AluOpType.add)
            nc.sync.dma_start(out=bn1i_d[:], in_=st1[:])

            nc.gpsimd.collective_compute(
                "AllReduce", mybir.AluOpType.add,
                ins=[bn1i_d[:]], outs=[bn1o_d[:]], replica_groups=RG,
            )
            st1g = l1.tile([128, 2], F32)
            nc.sync.dma_start(out=st1g[:], in_=bn1o_d[:])
            sc1, sh1 = _bn_apply_params(nc, l1, st1g, 0, N, g1_sb, be1_sb, "1")

            # apply BN1 + relu -> fp16, transpose to node-major, store
            h1T16 = l1.tile([H1, NB * 128], F16)
            for k in range(NKC):
                nc.scalar.activation(
                    out=h1T16[:, k * 512 : (k + 1) * 512],
                    in_=h1T_sb[:, k * 512 : (k + 1) * 512],
                    func=mybir.ActivationFunctionType.Relu,
                    bias=sh1[:], scale=sc1[:],
                )
            h1nm = l1.tile([128, NB, H1], F16)
            for t in range(NB):
                pt = ps.tile([128, 128], F16, tag="pt")
                nc.tensor.transpose(out=pt[:], in_=h1T16[:, t * 128 : (t + 1) * 128],
                                    identity=id16_sb[:])
                nc.vector.tensor_copy(out=h1nm[:, t, :], in_=pt[:])
            nc.sync.dma_start(
                out=h1loc_d[:].rearrange("(t p) f -> p t f", p=128), in_=h1nm[:]
            )

            nc.gpsimd.collective_compute(
                "AllGather", mybir.AluOpType.bypass,
                ins=[h1loc_d[:]], outs=[h1full_d[:]], replica_groups=RG,
            )
            l1s.release()
            l1.release()

            # ---------- layer 2 ----------
            l2 = tc.alloc_tile_pool(name="l2", bufs=1)
            l2s = tc.alloc_tile_pool(name="l2s", bufs=3)
            aggT2 = l2.tile([H1, NB * 128], F32)
            for b in range(NB):
                ohb = ohp.tile([128, C, 128], F16, tag="ohb2")
                nc.sync.dma_start(out=ohb[:], in_=oh_d[:, b * C : (b + 1) * C, :])
                hgb = l2s.tile([128, C, H1], F16, tag="hg")
                j0 = b * C
                for c0 in range(0, C, CCH):
                    nch = min(CCH, C - c0)
                    nc.gpsimd.dma_gather(
                        out_ap=hgb[:, c0 : c0 + nch, :],
                        in_ap=h1full_d[:],
                        idxs_ap=gidx_sb[:, (j0 + c0) * 8 : (j0 + c0 + nch) * 8],
                        num_idxs=nch * 128,
                        num_idxs_reg=nch * 128,
                        elem_size=H1,
                    )
                acc2 = psacc.tile([H1, 128], F32, tag="acc2")
                for cidx in range(C):
                    nc.tensor.matmul(
                        out=acc2[:],
                        lhsT=hgb[:, cidx, :],
                        rhs=ohb[:, cidx, :],
                        start=(cidx == 0),
                        stop=(cidx == C - 1),
                    )
                nc.vector.tensor_copy(out=aggT2[:, b * 128 : (b + 1) * 128], in_=acc2[:])

            # project halves: z2T [128, 2, PC] ; BN2 stats
            z2T = l2.tile([128, 2, NB * 128], F32)
            s2p = l2.tile([128, 2, 16], F32)
            for half in range(2):
                for k in range(NKC):
                    ph = ps.tile([128, 512], F32, tag="ph")
                    nc.tensor.matmul(
                        out=ph[:],
                        lhsT=w2_sb[:, half * 128 : (half + 1) * 128],
                        rhs=aggT2[:, k * 512 : (k + 1) * 512],
                        start=True, stop=True,
                    )
                    nc.vector.tensor_copy(out=z2T[:, half, k * 512 : (k + 1) * 512], in_=ph[:])
                    nc.vector.tensor_reduce(
                        out=s2p[:, half, k : k + 1], in_=ph[:],
                        axis=mybir.AxisListType.X, op=mybir.AluOpType.add,
                    )
                    sq = l2s.tile([128, 512], F32, tag="sq2")
                    nc.scalar.square(out=sq[:], in_=ph[:])
                    nc.vector.tensor_reduce(
                        out=s2p[:, half, NKC + k : NKC + k + 1], in_=sq[:],
                        axis=mybir.AxisListType.X, op=mybir.AluOpType.add,
                    )
            st2 = l2.tile([128, 4], F32)
            for half in range(2):
                nc.vector.tensor_reduce(out=st2[:, 2 * half : 2 * half + 1],
                                        in_=s2p[:, half, 0:NKC],
                                        axis=mybir.AxisListType.X, op=mybir.AluOpType.add)
                nc.vector.tensor_reduce(out=st2[:, 2 * half + 1 : 2 * half + 2],
                                        in_=s2p[:, half, NKC:2 * NKC],
                                        axis=mybir.AxisListType.X, op=mybir.AluOpType.add)
            nc.sync.dma_start(out=bn2i_d[:], in_=st2[:])
            nc.gpsimd.collective_compute(
                "AllReduce", mybir.AluOpType.add,
                ins=[bn2i_d[:]], outs=[bn2o_d[:]], replica_groups=RG,
            )
            st2g = l2.tile([128, 4], F32)
            nc.sync.dma_start(out=st2g[:], in_=bn2o_d[:])

            h2T16 = l2.tile([128, 2, NB * 128], F16)
            for half in range(2):
                sc2, sh2 = _bn_apply_params(
                    nc, l2, st2g, 2 * half, N,
                    g2_sb[:, half : half + 1], be2_sb[:, half : half + 1], "2",
                )
                for k in range(NKC):
                    nc.scalar.activation(
                        out=h2T16[:, half, k * 512 : (k + 1) * 512],
                        in_=z2T[:, half, k * 512 : (k + 1) * 512],
                        func=mybir.ActivationFunctionType.Relu,
                        bias=sh2[:], scale=sc2[:],
                    )
            # transpose to node-major
            h2nm = l2.tile([128, NB, 2, 128], F16)
            for t in range(NB):
                for half in range(2):
                    pt = ps.tile([128, 128], F16, tag="pt")
                    nc.tensor.transpose(out=pt[:], in_=h2T16[:, half, t * 128 : (t + 1) * 128],
                                        identity=id16_sb[:])
                    nc.vector.tensor_copy(out=h2nm[:, t, half, :], in_=pt[:])
            l2s.release()

            # ---------- mean-pool + head ----------
            tl = tc.alloc_tile_pool(name="tail", bufs=1)
            poolT = tl.tile([128, 2, G], F32)
            for half in range(2):
                pp = ps.tile([128, G], F32, tag="ph")
                for t in range(NB):
                    nc.tensor.matmul(
                        out=pp[:],
                        lhsT=h2nm[:, t, half, :],
                        rhs=ohg_sb[:, t, :],
                        start=(t == 0),
                        stop=(t == NB - 1),
                    )
                nc.vector.tensor_copy(out=poolT[:, half, :], in_=pp[:])
            nc.sync.dma_start(out=pli_d[:].rearrange("(h p) g -> p h g", p=128),
                              in_=poolT[:])
            nc.gpsimd.collective_compute(
                "AllReduce", mybir.AluOpType.add,
                ins=[pli_d[:]], outs=[plo_d[:]], replica_groups=RG,
            )
            poolTg = tl.tile([128, 2, G], F32)
            nc.sync.dma_start(out=poolTg[:],
                              in_=plo_d[:].rearrange("(h p) g -> p h g", p=128))

            # head
            psv = ps.tile([128, G], F32, tag="ph")
            nc.tensor.matmul(out=psv[:], lhsT=ws_sb[:], rhs=sfT_sb[:],
                             start=True, stop=True)
            solvT = tl.tile([128, G], F32)
            nc.scalar.activation(out=solvT[:], in_=psv[:],
                                 func=mybir.ActivationFunctionType.Relu,
                                 bias=bs_sb[:], scale=1.0)

            pzf = ps.tile([128, G], F32, tag="ph")
            zins = [poolTg[:, 0, :], poolTg[:, 1, :], solvT[:]]
            for k in range(3):
                nc.tensor.matmul(
                    out=pzf[:], lhsT=wf1_sb[:, k, :], rhs=zins[k],
                    start=(k == 0), stop=(k == 2),
                )
            zf_sb = tl.tile([128, G], F32)
            nc.vector.tensor_copy(out=zf_sb[:], in_=pzf[:])
            st3 = tl.tile([128, 2], F32)
            nc.vector.tensor_reduce(out=st3[:, 0:1], in_=zf_sb[:],
                                    axis=mybir.AxisListType.X, op=mybir.AluOpType.add)
            sq3 = tl.tile([128, G], F32)
            nc.scalar.square(out=sq3[:], in_=zf_sb[:])
            nc.vector.tensor_reduce(out=st3[:, 1:2], in_=sq3[:],
                                    axis=mybir.AxisListType.X, op=mybir.AluOpType.add)
            sc3, sh3 = _bn_apply_params(nc, tl, st3, 0, G, gf1_sb, bef1_sb, "3")
            zfa = tl.tile([128, G], F32)
            nc.scalar.activation(out=zfa[:], in_=zf_sb[:],
                                 func=mybir.ActivationFunctionType.Relu,
                                 bias=sh3[:], scale=sc3[:])

            po = ps.tile([1, G], F32, tag="ph")
            nc.tensor.matmul(out=po[:], lhsT=wf2_sb[:], rhs=zfa[:],
                             start=True, stop=True)
            out_sb = tl.tile([1, G], F32)
            nc.vector.tensor_scalar(
                out=out_sb[:], in0=po[:], scalar1=bf2_sb[:], scalar2=None,
                op0=mybir.AluOpType.add,
            )
            nc.sync.dma_start(out=out_d[:].rearrange("n o -> o n"), in_=out_sb[:])
            tl.release()
            l2.release()

    nc.finalize()
    _legalize_waits(nc)
    return nc


def _legalize_waits(nc, max_waits=1):
    """This walrus build rejects instructions with >1-2 sem waits. Hoist the
    excess onto preceding same-engine NoOps (sequencers run in program order)."""
    for fn in nc.m.functions:
        for bb in fn.blocks:
            new_insts = []
            for ins in bb.instructions:
                si = ins.sync_info
                if si is not None and si.on_wait and len(si.on_wait) > max_waits:
                    waits = list(si.on_wait)
                    keep = waits[: max_waits - 1] if max_waits > 1 else []
                    move = waits[len(keep):]
                    keep.append(move.pop())
                    for i, wv in enumerate(move):
                        nop = mybir.InstNoOp(name=f"{ins.name}_ws{i}", ins=[], outs=[],
                                             engine=ins.engine)
                        nop.sync_info = mybir.SyncInfo(on_wait=[wv], on_update=[])
                        new_insts.append(nop)
                        nc.register_instruction(nop, overwrite=True)
                    si.on_wait = keep
                new_insts.append(ins)
            bb.instructions[:] = new_insts


def _preprocess(x, edge_index, batch, solvent_fingerprint,
                W1, b1, g1, be1, W2, b2, g2, be2,
                Ws, bs, Wf1, bf1, gf1, bef1, Wf2, bf2):
    """Host-side sharding/index preprocessing. Returns (C, in_maps)."""
    edge_index = np.asarray(edge_index)
    batch = np.asarray(batch).astype(np.int64)
    x = np.ascontiguousarray(np.asarray(x, dtype=np.float32))

    loops = np.arange(N, dtype=np.int64)
    row = np.concatenate([edge_index[0].astype(np.int64), loops])
    col = np.concatenate([edge_index[1].astype(np.int64), loops])
    deg = np.bincount(col, minlength=N).astype(np.float32)
    dis = (1.0 / np.sqrt(deg)).astype(np.float32)
    norm = (dis[row] * dis[col]).astype(np.float32)

    perm = np.argsort(col, kind="stable")
    row, col, norm = row[perm], col[perm], norm[perm]

    gblk = col // 128
    nblk_tot = W * NB
    cnt = np.bincount(gblk, minlength=nblk_tot)
    C = int(np.ceil(cnt.max() / 128))
    NC = NB * C
    NE = NC * 128

    starts = np.zeros(nblk_tot + 1, np.int64)
    np.cumsum(cnt, out=starts[1:])
    rank = np.arange(row.shape[0]) - starts[gblk]

    # slot (core, blk, rank): chunk j = blk*C + rank//128, partition p = rank%128
    ridx = np.zeros((W, NB, C * 128), np.int16)
    core_of = gblk // NB
    blk_of = gblk % NB
    ridx[core_of, blk_of, rank] = row.astype(np.int16)

    # host-built one-hot (norm folded): oh[core, p, j, d]
    oh = np.zeros((W, NB, C * 128, 128), np.float16)
    oh[core_of, blk_of, rank, col % 128] = norm
    oh = oh.reshape(W, NB * C, 128, 128).transpose(0, 2, 1, 3).copy()

    # host-pregathered layer-1 rows: xg[core, p, j, :]
    x16 = x.astype(np.float16)
    xg = x16[ridx.reshape(W, NE)].reshape(W, NB * C, 128, F_IN)
    xg = xg.transpose(0, 2, 1, 3).copy()

    # pool one-hot with 1/count folded: ohg[core, p, t, g]
    gcnt = np.bincount(batch, minlength=G).astype(np.float32)
    node_w = np.zeros(NP, np.float32)
    node_w[:N] = (1.0 / np.maximum(gcnt, 1.0))[batch]
    node_g = np.zeros(NP, np.int64)
    node_g[:N] = batch
    ohg = np.zeros((NP, G), np.float16)
    ohg[np.arange(NP), node_g] = node_w
    ohg = ohg.reshape(W, NB, 128, G).transpose(0, 2, 1, 3).copy()

    rep = {
        "ident16": np.eye(128, dtype=np.float16),
        "sfT": np.ascontiguousarray(np.asarray(solvent_fingerprint, np.float32).T),
        "w1": np.asarray(W1, np.float32), "w2": np.asarray(W2, np.float32),
        "ws": np.asarray(Ws, np.float32),
        "wf1": np.ascontiguousarray(
            np.asarray(Wf1, np.float32).reshape(3, 128, 128).transpose(1, 0, 2)),
        "wf2": np.asarray(Wf2, np.float32).reshape(128, 1),
        "g1": np.asarray(g1, np.float32).reshape(128, 1),
        "be1": np.asarray(be1, np.float32).reshape(128, 1),
        "g2": np.ascontiguousarray(np.asarray(g2, np.float32).reshape(2, 128).T),
        "be2": np.ascontiguousarray(np.asarray(be2, np.float32).reshape(2, 128).T),
        "gf1": np.asarray(gf1, np.float32).reshape(128, 1),
        "bef1": np.asarray(bef1, np.float32).reshape(128, 1),
        "bs": np.asarray(bs, np.float32).reshape(128, 1),
        "bf2": np.asarray(bf2, np.float32).reshape(1, 1),
    }

    in_maps = []
    for c in range(W):
        r = ridx[c].reshape(NE)
        gidx = np.tile(r.reshape(NE // 16, 16).T, (8, 1)).copy()
        # indirect-DMA index layout: idx2[p, j] = source row of slot (j, p)
        idx2 = np.ascontiguousarray(r.reshape(NB * C, 128).T).astype(np.int32)
        m = dict(rep)
        m.update({
            "xg": xg[c],
            "oh": oh[c],
            "gidx": gidx,
            "idx2": idx2,
            "ohg": ohg[c],
        })
        in_maps.append(m)
    return C, in_maps


_PROG_CACHE = {}


def _get_program(C):
    if C not in _PROG_CACHE:
        _PROG_CACHE[C] = _build_program(C)
    return _PROG_CACHE[C]


def kernel(**inputs) -> np.ndarray:
    C, in_maps = _preprocess(**inputs)
    nc = _get_program(C)
    res = run_bass_kernel_spmd(nc, in_maps, core_ids=list(range(W)))
    return np.asarray(res.results[0]["out"], dtype=np.float32)


# revision 26
# speedup vs baseline: 1.0591x; 1.0591x over previous
"""Trainium2 Bass kernel for ChromophoreSolventGNN (2x GCNConv + BN + mean-pool + MLP head).

Strategy (8 NeuronCores, SPMD), v2:
  - Destination-shard nodes: core c owns contiguous node range [c*2560, (c+1)*2560)
    (N=20000 padded to 20480). Edges (incl. self-loops) routed to the owner of
    their destination (col), sorted by destination block (128 nodes).
  - Layer-1 edge source rows are PRE-GATHERED ON HOST (xg, fp16) - no device
    gather for layer 1. One-hot scatter matrices (norm folded in) and the
    mean-pool one-hot are also host-built and streamed from HBM, eliminating
    the DVE one-hot generation of v1.
  - Aggregation computes agg^T directly: matmul(lhsT=rows[128e,F], rhs=oh[128e,128d])
    -> psum [F, 128d]; feature-major feeds the projection without transposes.
  - Layer-2 gathers h1 rows from the AllGathered table via gpsimd dma_gather
    (small per-call descriptor prep, overlapped with compute via buffered pools).
  - BatchNorm is shift-invariant => conv biases drop out. Stats AllReduce'd.
  - Mean-pool via one-hot matmul; pooled sums AllReduce'd; small MLP head
    computed replicated on every core.
"""

import numpy as np

import concourse.bass as bass
import concourse.mybir as mybir
from concourse import bacc
from concourse.bass_utils import run_bass_kernel_spmd
from concourse.tile import TileContext

F32 = mybir.dt.float32
F16 = mybir.dt.float16
I16 = mybir.dt.int16
I32 = mybir.dt.int32

W = 8            # cores
N = 20000        # nodes
E = 320000       # edges
G = 512          # graphs
F_IN = 64
H1 = 128
H2 = 256
SOLV = 128
EPS = 1e-5

NB = 20                  # destination blocks of 128 nodes per core
PC = NB * 128            # nodes per core (2560)
NP = W * PC              # padded node count (20480)
CCH_MAX = 8              # chunks per gather call (<=1024 idxs, HW desc-ring limit)


def _bn_apply_params(nc, tl, st, colw, n_count, g_sb, be_sb, name):
    """From (sum, sumsq) slices compute per-partition scale/shift tiles."""
    mu = tl.tile([128, 1], F32, tag=f"mu{name}")
    nc.vector.tensor_scalar_mul(mu[:], st[:, colw : colw + 1], 1.0 / n_count)
    var = tl.tile([128, 1], F32, tag=f"var{name}")
    nc.vector.tensor_scalar_mul(var[:], st[:, colw + 1 : colw + 2], 1.0 / n_count)
    musq = tl.tile([128, 1], F32, tag=f"musq{name}")
    nc.vector.tensor_tensor(out=musq[:], in0=mu[:], in1=mu[:], op=mybir.AluOpType.mult)
    nc.vector.tensor_tensor(out=var[:], in0=var[:], in1=musq[:], op=mybir.AluOpType.subtract)
    nc.vector.tensor_scalar_add(var[:], var[:], EPS)
    rv = tl.tile([128, 1], F32, tag=f"rv{name}")
    nc.vector.reciprocal(out=rv[:], in_=var[:])
    rstd = tl.tile([128, 1], F32, tag=f"rstd{name}")
    nc.scalar.sqrt(out=rstd[:], in_=rv[:])
    sc = tl.tile([128, 1], F32, tag=f"sc{name}")
    nc.vector.tensor_tensor(out=sc[:], in0=g_sb[:], in1=rstd[:], op=mybir.AluOpType.mult)
    sh = tl.tile([128, 1], F32, tag=f"sh{name}")
    nc.vector.tensor_tensor(out=sh[:], in0=mu[:], in1=sc[:], op=mybir.AluOpType.mult)
    nc.vector.tensor_tensor(out=sh[:], in0=be_sb[:], in1=sh[:], op=mybir.AluOpType.subtract)
    return sc, sh


def _build_program(C):
    """Build the SPMD Bass program. C = chunks (of 128 edge slots) per dst block."""
    NC = NB * C           # chunks per core
    NE = NC * 128         # edge slots per core

    nc = bacc.Bacc("TRN2", target_bir_lowering=False, debug=False, num_devices=W)

    # ---- external inputs -------------------------------------------------
    xg_d = nc.dram_tensor("xg", [128, NC, F_IN], F16, kind="ExternalInput")
    oh_d = nc.dram_tensor("oh", [128, NC, 128], F16, kind="ExternalInput")
    gidx_d = nc.dram_tensor("gidx", [128, NE // 16], I16, kind="ExternalInput")
    ohg_d = nc.dram_tensor("ohg", [128, NB, G], F16, kind="ExternalInput")
    id16_d = nc.dram_tensor("ident16", [128, 128], F16, kind="ExternalInput")
    sfT_d = nc.dram_tensor("sfT", [SOLV, G], F32, kind="ExternalInput")
    w1_d = nc.dram_tensor("w1", [F_IN, H1], F32, kind="ExternalInput")
    w2_d = nc.dram_tensor("w2", [H1, H2], F32, kind="ExternalInput")
    ws_d = nc.dram_tensor("ws", [SOLV, 128], F32, kind="ExternalInput")
    wf1_d = nc.dram_tensor("wf1", [128, 3, 128], F32, kind="ExternalInput")
    wf2_d = nc.dram_tensor("wf2", [128, 1], F32, kind="ExternalInput")
    g1_d = nc.dram_tensor("g1", [128, 1], F32, kind="ExternalInput")
    be1_d = nc.dram_tensor("be1", [128, 1], F32, kind="ExternalInput")
    g2_d = nc.dram_tensor("g2", [128, 2], F32, kind="ExternalInput")
    be2_d = nc.dram_tensor("be2", [128, 2], F32, kind="ExternalInput")
    gf1_d = nc.dram_tensor("gf1", [128, 1], F32, kind="ExternalInput")
    bef1_d = nc.dram_tensor("bef1", [128, 1], F32, kind="ExternalInput")
    bs_d = nc.dram_tensor("bs", [128, 1], F32, kind="ExternalInput")
    bf2_d = nc.dram_tensor("bf2", [1, 1], F32, kind="ExternalInput")

    out_d = nc.dram_tensor("out", [G, 1], F32, kind="ExternalOutput")

    # ---- internal DRAM ---------------------------------------------------
    h1loc_d = nc.dram_tensor("h1loc", [PC, H1], F16)
    h1full_d = nc.dram_tensor("h1full", [NP, H1], F16, addr_space="Shared")
    bn1i_d = nc.dram_tensor("bn1i", [128, 2], F32)
    bn1o_d = nc.dram_tensor("bn1o", [W * 128, 2], F32, addr_space="Shared")
    bn2i_d = nc.dram_tensor("bn2i", [128, 4], F32)
    bn2o_d = nc.dram_tensor("bn2o", [W * 128, 4], F32, addr_space="Shared")
    pli_d = nc.dram_tensor("pli", [2 * 128, G], F32)
    plo_d = nc.dram_tensor("plo", [2 * 128, G], F32, addr_space="Shared")

    RG = [list(range(W))]
    CCH = CCH_MAX
    while NC % CCH:
        CCH -= 1
    NGC = NC // CCH        # gather calls / stream groups per layer
    CPC = CCH * 128        # edge slots per gather call (<=1024)
    NKC = PC // 512        # 512-node column chunks

    with TileContext(nc) as tc:
        with tc.tile_pool(name="const", bufs=1) as cst, \
             tc.tile_pool(name="ohst", bufs=3) as ohp, \
             tc.tile_pool(name="ps", bufs=2, space="PSUM") as ps, \
             tc.tile_pool(name="psacc", bufs=2, space="PSUM") as psacc:

            # ---------- setup: constants ----------
            def load_const(name, dram, shape, dt):
                t = cst.tile(shape, dt, name=name)
                nc.sync.dma_start(out=t[:], in_=dram[:])
                return t

            id16_sb = load_const("id16_sb", id16_d, [128, 128], F16)
            sfT_sb = load_const("sfT_sb", sfT_d, [SOLV, G], F32)
            w1_sb = load_const("w1_sb", w1_d, [F_IN, H1], F32)
            w2_sb = load_const("w2_sb", w2_d, [H1, H2], F32)
            ws_sb = load_const("ws_sb", ws_d, [SOLV, 128], F32)
            wf1_sb = load_const("wf1_sb", wf1_d, [128, 3, 128], F32)
            wf2_sb = load_const("wf2_sb", wf2_d, [128, 1], F32)
            g1_sb = load_const("g1_sb", g1_d, [128, 1], F32)
            be1_sb = load_const("be1_sb", be1_d, [128, 1], F32)
            g2_sb = load_const("g2_sb", g2_d, [128, 2], F32)
            be2_sb = load_const("be2_sb", be2_d, [128, 2], F32)
            gf1_sb = load_const("gf1_sb", gf1_d, [128, 1], F32)
            bef1_sb = load_const("bef1_sb", bef1_d, [128, 1], F32)
            bs_sb = load_const("bs_sb", bs_d, [128, 1], F32)
            bf2_sb = load_const("bf2_sb", bf2_d, [1, 1], F32)

            # ---------- layer 1: agg^T = (Xg)^T-chunks @ oh-chunks ----------
            # Blocks are processed in pairs with interleaved matmuls so two
            # independent PSUM accumulation chains pipeline on the PE array.
            # The projection of each finished 512-column slice (4 blocks) is
            # interleaved so BN1 stats complete right after the last block.
            l1 = tc.alloc_tile_pool(name="l1", bufs=1)
            l1s = tc.alloc_tile_pool(name="l1s", bufs=4)
            aggT1 = l1.tile([F_IN, NB * 128], F32)
            h1T_sb = l1.tile([H1, NB * 128], F32)
            s1p = l1.tile([128, 16], F32)

            def emit_proj1(k):
                ph = ps.tile([H1, 512], F32, tag="ph")
                nc.tensor.matmul(
                    out=ph[:], lhsT=w1_sb[:],
                    rhs=aggT1[:, k * 512 : (k + 1) * 512],
                    start=True, stop=True,
                )
                nc.vector.tensor_copy(out=h1T_sb[:, k * 512 : (k + 1) * 512], in_=ph[:])
                nc.vector.tensor_reduce(
                    out=s1p[:, k : k + 1], in_=ph[:],
                    axis=mybir.AxisListType.X, op=mybir.AluOpType.add,
                )
                sq = l1s.tile([H1, 512], F32, tag="sq1")
                nc.scalar.square(out=sq[:], in_=ph[:])
                nc.vector.tensor_reduce(
                    out=s1p[:, NKC + k : NKC + k + 1], in_=sq[:],
                    axis=mybir.AxisListType.X, op=mybir.AluOpType.add,
                )

            for b0 in range(0, NB, 2):
                tiles = []
                for b in (b0, b0 + 1):
                    ohb = ohp.tile([128, C, 128], F16, tag="ohb")
                    nc.sync.dma_start(out=ohb[:], in_=oh_d[:, b * C : (b + 1) * C, :])
                    xgb = l1s.tile([128, C, F_IN], F16, tag="xgb")
                    nc.sync.dma_start(out=xgb[:], in_=xg_d[:, b * C : (b + 1) * C, :])
                    acc = psacc.tile([F_IN, 128], F32, tag="acc")
                    tiles.append((ohb, xgb, acc))
                for cidx in range(C):
                    for ohb, xgb, acc in tiles:
                        nc.tensor.matmul(
                            out=acc[:],
                            lhsT=xgb[:, cidx, :],
                            rhs=ohb[:, cidx, :],
                            start=(cidx == 0),
                            stop=(cidx == C - 1),
                        )
                for i, (ohb, xgb, acc) in enumerate(tiles):
                    nc.vector.tensor_copy(
                        out=aggT1[:, (b0 + i) * 128 : (b0 + i + 1) * 128], in_=acc[:]
                    )
                if (b0 + 2) % 4 == 0:
                    emit_proj1((b0 + 2) // 4 - 1)

            # deferred constants (first needed in layer 2 / pool)
            gidx_sb = load_const("gidx_sb", gidx_d, [128, NE // 16], I16)
            ohg_sb = load_const("ohg_sb", ohg_d, [128, NB, G], F16)

            st1 = l1.tile([128, 2], F32)
            nc.vector.tensor_reduce(out=st1[:, 0:1], in_=s1p[:, 0:NKC],
                                    axis=mybir.AxisListType.X, op=mybir.AluOpType.add)
            nc.vector.tensor_reduce(out=st1[:, 1:2], in_=s1p[:, NKC:2 * NKC],
                                    axis=mybir.AxisListType.X, op=mybir.AluOpType.add)
            nc.sync.dma_start(out=bn1i_d[:], in_=st1[:])

            nc.gpsimd.collective_compute(
                "AllGather", mybir.AluOpType.bypass,
                ins=[bn1i_d[:]], outs=[bn1o_d[:]], replica_groups=RG,
            )
            st1g8 = l1.tile([128, 2, W], F32)
            nc.sync.dma_start(out=st1g8[:],
                              in_=bn1o_d[:].rearrange("(w p) s -> p s w", p=128))
            st1g = l1.tile([128, 2], F32)
            for s in range(2):
                nc.vector.tensor_reduce(out=st1g[:, s : s + 1], in_=st1g8[:, s, :],
                                        axis=mybir.AxisListType.X, op=mybir.AluOpType.add)
            sc1, sh1 = _bn_apply_params(nc, l1, st1g, 0, N, g1_sb, be1_sb, "1")

            # apply BN1 + relu -> fp16, transpose to node-major, store
            h1T16 = l1.tile([H1, NB * 128], F16)
            for k in range(NKC):
                nc.scalar.activation(
                    out=h1T16[:, k * 512 : (k + 1) * 512],
                    in_=h1T_sb[:, k * 512 : (k + 1) * 512],
                    func=mybir.ActivationFunctionType.Relu,
                    bias=sh1[:], scale=sc1[:],
                )
            h1nm = l1.tile([128, NB, H1], F16)
            for t in range(NB):
                pt = ps.tile([128, 128], F16, tag="pt")
                nc.tensor.transpose(out=pt[:], in_=h1T16[:, t * 128 : (t + 1) * 128],
                                    identity=id16_sb[:])
                nc.vector.tensor_copy(out=h1nm[:, t, :], in_=pt[:])
            nc.sync.dma_start(
                out=h1loc_d[:].rearrange("(t p) f -> p t f", p=128), in_=h1nm[:]
            )

            nc.gpsimd.collective_compute(
                "AllGather", mybir.AluOpType.bypass,
                ins=[h1loc_d[:]], outs=[h1full_d[:]], replica_groups=RG,
            )
            l1s.release()
            l1.release()

            # ---------- layer 2 ----------
            l2 = tc.alloc_tile_pool(name="l2", bufs=1)
            l2s = tc.alloc_tile_pool(name="l2s", bufs=3)
            aggT2 = l2.tile([H1, NB * 128], F32)
            z2T = l2.tile([128, 2, NB * 128], F32)
            s2p = l2.tile([128, 2, 16], F32)

            def emit_proj2(k):
                for half in range(2):
                    ph = ps.tile([128, 512], F32, tag="ph")
                    nc.tensor.matmul(
                        out=ph[:],
                        lhsT=w2_sb[:, half * 128 : (half + 1) * 128],
                        rhs=aggT2[:, k * 512 : (k + 1) * 512],
                        start=True, stop=True,
                    )
                    nc.vector.tensor_copy(out=z2T[:, half, k * 512 : (k + 1) * 512], in_=ph[:])
                    nc.vector.tensor_reduce(
                        out=s2p[:, half, k : k + 1], in_=ph[:],
                        axis=mybir.AxisListType.X, op=mybir.AluOpType.add,
                    )
                    sq = l2s.tile([128, 512], F32, tag="sq2")
                    nc.scalar.square(out=sq[:], in_=ph[:])
                    nc.vector.tensor_reduce(
                        out=s2p[:, half, NKC + k : NKC + k + 1], in_=sq[:],
                        axis=mybir.AxisListType.X, op=mybir.AluOpType.add,
                    )

            for b in range(NB):
                ohb = ohp.tile([128, C, 128], F16, tag="ohb2")
                nc.sync.dma_start(out=ohb[:], in_=oh_d[:, b * C : (b + 1) * C, :])
                hgb = l2s.tile([128, C, H1], F16, tag="hg")
                j0 = b * C
                for c0 in range(0, C, CCH):
                    nch = min(CCH, C - c0)
                    nc.gpsimd.dma_gather(
                        out_ap=hgb[:, c0 : c0 + nch, :],
                        in_ap=h1full_d[:],
                        idxs_ap=gidx_sb[:, (j0 + c0) * 8 : (j0 + c0 + nch) * 8],
                        num_idxs=nch * 128,
                        num_idxs_reg=nch * 128,
                        elem_size=H1,
                    )
                acc2 = psacc.tile([H1, 128], F32, tag="acc2")
                for cidx in range(C):
                    nc.tensor.matmul(
                        out=acc2[:],
                        lhsT=hgb[:, cidx, :],
                        rhs=ohb[:, cidx, :],
                        start=(cidx == 0),
                        stop=(cidx == C - 1),
                    )
                nc.vector.tensor_copy(out=aggT2[:, b * 128 : (b + 1) * 128], in_=acc2[:])
                if (b + 1) % 4 == 0:
                    emit_proj2((b + 1) // 4 - 1)

            st2 = l2.tile([128, 4], F32)
            for half in range(2):
                nc.vector.tensor_reduce(out=st2[:, 2 * half : 2 * half + 1],
                                        in_=s2p[:, half, 0:NKC],
                                        axis=mybir.AxisListType.X, op=mybir.AluOpType.add)
                nc.vector.tensor_reduce(out=st2[:, 2 * half + 1 : 2 * half + 2],
                                        in_=s2p[:, half, NKC:2 * NKC],
                                        axis=mybir.AxisListType.X, op=mybir.AluOpType.add)
            nc.sync.dma_start(out=bn2i_d[:], in_=st2[:])
            nc.gpsimd.collective_compute(
                "AllGather", mybir.AluOpType.bypass,
                ins=[bn2i_d[:]], outs=[bn2o_d[:]], replica_groups=RG,
            )
            st2g8 = l2.tile([128, 4, W], F32)
            nc.sync.dma_start(out=st2g8[:],
                              in_=bn2o_d[:].rearrange("(w p) s -> p s w", p=128))
            st2g = l2.tile([128, 4], F32)
            for s in range(4):
                nc.vector.tensor_reduce(out=st2g[:, s : s + 1], in_=st2g8[:, s, :],
                                        axis=mybir.AxisListType.X, op=mybir.AluOpType.add)

            h2T16 = l2.tile([128, 2, NB * 128], F16)
            for half in range(2):
                sc2, sh2 = _bn_apply_params(
                    nc, l2, st2g, 2 * half, N,
                    g2_sb[:, half : half + 1], be2_sb[:, half : half + 1], "2",
                )
                for k in range(NKC):
                    nc.scalar.activation(
                        out=h2T16[:, half, k * 512 : (k + 1) * 512],
                        in_=z2T[:, half, k * 512 : (k + 1) * 512],
                        func=mybir.ActivationFunctionType.Relu,
                        bias=sh2[:], scale=sc2[:],
                    )
            # transpose to node-major
            h2nm = l2.tile([128, NB, 2, 128], F16)
            for t in range(NB):
                for half in range(2):
                    pt = ps.tile([128, 128], F16, tag="pt")
                    nc.tensor.transpose(out=pt[:], in_=h2T16[:, half, t * 128 : (t + 1) * 128],
                                        identity=id16_sb[:])
                    nc.vector.tensor_copy(out=h2nm[:, t, half, :], in_=pt[:])
            l2s.release()

            # ---------- mean-pool + head ----------
            tl = tc.alloc_tile_pool(name="tail", bufs=1)
            poolT = tl.tile([128, 2, G], F32)
            for half in range(2):
                pp = ps.tile([128, G], F32, tag="ph")
                for t in range(NB):
                    nc.tensor.matmul(
                        out=pp[:],
                        lhsT=h2nm[:, t, half, :],
                        rhs=ohg_sb[:, t, :],
                        start=(t == 0),
                        stop=(t == NB - 1),
                    )
                nc.vector.tensor_copy(out=poolT[:, half, :], in_=pp[:])
            nc.sync.dma_start(out=pli_d[:].rearrange("(h p) g -> p h g", p=128),
                              in_=poolT[:])
            nc.gpsimd.collective_compute(
                "AllReduce", mybir.AluOpType.add,
                ins=[pli_d[:]], outs=[plo_d[:]], replica_groups=RG,
            )

            # solvent head is independent of the pool - hides under the AllReduce
            psv = ps.tile([128, G], F32, tag="ph")
            nc.tensor.matmul(out=psv[:], lhsT=ws_sb[:], rhs=sfT_sb[:],
                             start=True, stop=True)
            solvT = tl.tile([128, G], F32)
            nc.scalar.activation(out=solvT[:], in_=psv[:],
                                 func=mybir.ActivationFunctionType.Relu,
                                 bias=bs_sb[:], scale=1.0)

            poolTg = tl.tile([128, 2, G], F32)
            nc.sync.dma_start(out=poolTg[:],
                              in_=plo_d[:].rearrange("(h p) g -> p h g", p=128))

            pzf = ps.tile([128, G], F32, tag="ph")
            zins = [poolTg[:, 0, :], poolTg[:, 1, :], solvT[:]]
            for k in range(3):
                nc.tensor.matmul(
                    out=pzf[:], lhsT=wf1_sb[:, k, :], rhs=zins[k],
                    start=(k == 0), stop=(k == 2),
                )
            zf_sb = tl.tile([128, G], F32)
            nc.vector.tensor_copy(out=zf_sb[:], in_=pzf[:])
            st3 = tl.tile([128, 2], F32)
            nc.vector.tensor_reduce(out=st3[:, 0:1], in_=zf_sb[:],
                                    axis=mybir.AxisListType.X, op=mybir.AluOpType.add)
            sq3 = tl.tile([128, G], F32)
            nc.scalar.square(out=sq3[:], in_=zf_sb[:])
            nc.vector.tensor_reduce(out=st3[:, 1:2], in_=sq3[:],
                                    axis=mybir.AxisListType.X, op=mybir.AluOpType.add)
            sc3, sh3 = _bn_apply_params(nc, tl, st3, 0, G, gf1_sb, bef1_sb, "3")
            zfa = tl.tile([128, G], F32)
            nc.scalar.activation(out=zfa[:], in_=zf_sb[:],
                                 func=mybir.ActivationFunctionType.Relu,
                                 bias=sh3[:], scale=sc3[:])

            po = ps.tile([1, G], F32, tag="ph")
            nc.tensor.matmul(out=po[:], lhsT=wf2_sb[:], rhs=zfa[:],
                             start=True, stop=True)
            out_sb = tl.tile([1, G], F32)
            nc.vector.tensor_scalar(
                out=out_sb[:], in0=po[:], scalar1=bf2_sb[:], scalar2=None,
                op0=mybir.AluOpType.add,
            )
            nc.sync.dma_start(out=out_d[:].rearrange("n o -> o n"), in_=out_sb[:])
            tl.release()
            l2.release()

    nc.finalize()
    _legalize_waits(nc)
    return nc


def _legalize_waits(nc, max_waits=1):
    """This walrus build rejects instructions with >1-2 sem waits. Hoist the
    excess onto preceding same-engine NoOps (sequencers run in program order)."""
    for fn in nc.m.functions:
        for bb in fn.blocks:
            new_insts = []
            for ins in bb.instructions:
                si = ins.sync_info
                if si is not None and si.on_wait and len(si.on_wait) > max_waits:
                    waits = list(si.on_wait)
                    keep = waits[: max_waits - 1] if max_waits > 1 else []
                    move = waits[len(keep):]
                    keep.append(move.pop())
                    for i, wv in enumerate(move):
                        nop = mybir.InstNoOp(name=f"{ins.name}_ws{i}", ins=[], outs=[],
                                             engine=ins.engine)
                        nop.sync_info = mybir.SyncInfo(on_wait=[wv], on_update=[])
                        new_insts.append(nop)
                        nc.register_instruction(nop, overwrite=True)
                    si.on_wait = keep
                new_insts.append(ins)
            bb.instructions[:] = new_insts


def _preprocess(x, edge_index, batch, solvent_fingerprint,
                W1, b1, g1, be1, W2, b2, g2, be2,
                Ws, bs, Wf1, bf1, gf1, bef1, Wf2, bf2):
    """Host-side sharding/index preprocessing. Returns (C, in_maps)."""
    edge_index = np.asarray(edge_index)
    batch = np.asarray(batch).astype(np.int64)
    x = np.ascontiguousarray(np.asarray(x, dtype=np.float32))

    loops = np.arange(N, dtype=np.int64)
    row = np.concatenate([edge_index[0].astype(np.int64), loops])
    col = np.concatenate([edge_index[1].astype(np.int64), loops])
    deg = np.bincount(col, minlength=N).astype(np.float32)
    dis = (1.0 / np.sqrt(deg)).astype(np.float32)
    norm = (dis[row] * dis[col]).astype(np.float32)

    perm = np.argsort(col, kind="stable")
    row, col, norm = row[perm], col[perm], norm[perm]

    gblk = col // 128
    nblk_tot = W * NB
    cnt = np.bincount(gblk, minlength=nblk_tot)
    C = int(np.ceil(cnt.max() / 128))
    NC = NB * C
    NE = NC * 128

    starts = np.zeros(nblk_tot + 1, np.int64)
    np.cumsum(cnt, out=starts[1:])
    rank = np.arange(row.shape[0]) - starts[gblk]

    # slot (core, blk, rank): chunk j = blk*C + rank//128, partition p = rank%128
    ridx = np.zeros((W, NB, C * 128), np.int16)
    core_of = gblk // NB
    blk_of = gblk % NB
    ridx[core_of, blk_of, rank] = row.astype(np.int16)

    # host-built one-hot (norm folded): oh[core, p, j, d]
    oh = np.zeros((W, NB, C * 128, 128), np.float16)
    oh[core_of, blk_of, rank, col % 128] = norm
    oh = oh.reshape(W, NB * C, 128, 128).transpose(0, 2, 1, 3).copy()

    # host-pregathered layer-1 rows: xg[core, p, j, :]
    x16 = x.astype(np.float16)
    xg = x16[ridx.reshape(W, NE)].reshape(W, NB * C, 128, F_IN)
    xg = xg.transpose(0, 2, 1, 3).copy()

    # pool one-hot with 1/count folded: ohg[core, p, t, g]
    gcnt = np.bincount(batch, minlength=G).astype(np.float32)
    node_w = np.zeros(NP, np.float32)
    node_w[:N] = (1.0 / np.maximum(gcnt, 1.0))[batch]
    node_g = np.zeros(NP, np.int64)
    node_g[:N] = batch
    ohg = np.zeros((NP, G), np.float16)
    ohg[np.arange(NP), node_g] = node_w
    ohg = ohg.reshape(W, NB, 128, G).transpose(0, 2, 1, 3).copy()

    rep = {
        "ident16": np.eye(128, dtype=np.float16),
        "sfT": np.ascontiguousarray(np.asarray(solvent_fingerprint, np.float32).T),
        "w1": np.asarray(W1, np.float32), "w2": np.asarray(W2, np.float32),
        "ws": np.asarray(Ws, np.float32),
        "wf1": np.ascontiguousarray(
            np.asarray(Wf1, np.float32).reshape(3, 128, 128).transpose(1, 0, 2)),
        "wf2": np.asarray(Wf2, np.float32).reshape(128, 1),
        "g1": np.asarray(g1, np.float32).reshape(128, 1),
        "be1": np.asarray(be1, np.float32).reshape(128, 1),
        "g2": np.ascontiguousarray(np.asarray(g2, np.float32).reshape(2, 128).T),
        "be2": np.ascontiguousarray(np.asarray(be2, np.float32).reshape(2, 128).T),
        "gf1": np.asarray(gf1, np.float32).reshape(128, 1),
        "bef1": np.asarray(bef1, np.float32).reshape(128, 1),
        "bs": np.asarray(bs, np.float32).reshape(128, 1),
        "bf2": np.asarray(bf2, np.float32).reshape(1, 1),
    }

    in_maps = []
    for c in range(W):
        r = ridx[c].reshape(NE)
        gidx = np.tile(r.reshape(NE // 16, 16).T, (8, 1)).copy()
        m = dict(rep)
        m.update({
            "xg": xg[c],
            "oh": oh[c],
            "gidx": gidx,
            "ohg": ohg[c],
        })
        in_maps.append(m)
    return C, in_maps


_PROG_CACHE = {}


def _get_program(C):
    if C not in _PROG_CACHE:
        _PROG_CACHE[C] = _build_program(C)
    return _PROG_CACHE[C]


def kernel(**inputs) -> np.ndarray:
    C, in_maps = _preprocess(**inputs)
    nc = _get_program(C)
    res = run_bass_kernel_spmd(nc, in_maps, core_ids=list(range(W)))
    return np.asarray(res.results[0]["out"], dtype=np.float32)
